# revision 13
# baseline (speedup 1.0000x reference)
"""Trainium2 Bass kernel for nn_BigAttention (weight-norm MLP + softmax-over-k).

Math (per the reference):
    W1e = g1 * W1 / ||W1||_F          [1024, 3072]
    W2e = g2 * W2 / ||W2||_F          [1, 1024]
    hv  = v @ W1e[:, :2048].T         [B,K,N,1024]
    hq  = q @ W1e[:, 2048:].T         [B,K,1024]
    joint  = relu(hv + hq + b1)
    logits = joint @ W2e.T  (+ b2, which cancels in the softmax over k)
    out = softmax(logits, axis=K)     [B,K,N,1]

Sharding: data-parallel over batch, 8 batches per core; weights replicated.

Per-core device program (rows r = (b_local, k, n) flattened, R = 8*12*36 = 3456):
  - hq[96, 1024] via PE (q^T chunks stationary), b1 folded in as a K=1 matmul.
  - main: per 128-row tile, PSUM[row, hidden 1024] accumulates 16 v^T-chunk
    matmuls (float32r: 1 cycle/row vs 4 for fp32) plus ONE one-hot matmul
    that adds hq[bk(row), :] (one-hot selection stationary, hq moving).
  - epilogue per tile: one DVE scalar_tensor_tensor computes
    (PSUM max 0) * w2_broadcast with accum_out = per-row sum = the logit.
  - softmax over k: logits go [128, 27] -> StreamTranspose -> linear DRAM ->
    [96 (b,k), 36 n] SBUF; exp on ACT; the per-(b,n) sum and its broadcast
    back over k are two tiny one-hot matmuls on the PE; final scale on DVE;
    one strided DMA writes the [8,12,36,1] output slice.

All heavy inputs are host-repacked "partition-major" so every big DMA is 128
contiguous runs (one per partition) instead of thousands of thin descriptors.
Weight DMAs ride the scalar-engine HWDGE ring, v DMAs the sync ring, tiny
constants the gpsimd SWDGE path, so descriptor generation overlaps.
"""

import numpy as np

import concourse.bacc as bacc
import concourse.mybir as mybir
import concourse.tile as tile
from concourse.bass_utils import run_bass_kernel_spmd

F32 = mybir.dt.float32
NCORES = 8
B, K, N = 64, 12, 36
VD, QD, HID = 2048, 1024, 1024
BL = B // NCORES              # local batches per core
R = BL * K * N                # 3456 rows per core
BK = BL * K                   # 96 (b,k) groups per core
CC = VD // 128                # 16 contraction chunks over v-dim
QC = QD // 128                # 8 contraction chunks over q-dim
RC = 384                      # rows per DMA chunk (9 chunks)
NCH = R // RC
RT = 128                      # rows per PSUM tile
NT = RC // RT
NRT = R // RT                 # 27 row tiles
VSPLIT = 4                    # v-chunk DMA granularity (cc chunks per DMA)

_NC_CACHE = None

# fp32 matmul runs at 4 cycles/row on the PE (decomposed into 2 half-rate
# passes); float32r (same 4-byte data, relaxed-precision multiply) streams at
# 1 cycle/row when the moving free dim is >= 256.
MM_DT = mybir.dt.float32r


def _build_nc():
    nc = bacc.Bacc("TRN2", target_bir_lowering=False, debug=False,
                   num_devices=NCORES)

    def mm(out, lhsT, rhs, **kw):
        nc.tensor.matmul(out, lhsT, rhs, **kw)

    vt = nc.dram_tensor("vt", [NCH, 128, CC, RC], MM_DT, kind="ExternalInput").ap()
    qt = nc.dram_tensor("qt", [128, QC, BK], MM_DT, kind="ExternalInput").ap()
    w1vt = nc.dram_tensor("w1vt", [128, CC, HID], MM_DT, kind="ExternalInput").ap()
    w1qt = nc.dram_tensor("w1qt", [128, QC, HID], MM_DT, kind="ExternalInput").ap()
    w2b = nc.dram_tensor("w2b", [128, HID], F32, kind="ExternalInput").ap()
    b1r = nc.dram_tensor("b1r", [1, HID], MM_DT, kind="ExternalInput").ap()
    ones = nc.dram_tensor("ones", [1, BK], MM_DT, kind="ExternalInput").ap()
    oneh = nc.dram_tensor("oneh", [BK, R], MM_DT, kind="ExternalInput").ap()
    selb = nc.dram_tensor("selb", [BK, BL], F32, kind="ExternalInput").ap()
    selbt = nc.dram_tensor("selbt", [BL, BK], F32, kind="ExternalInput").ap()
    out = nc.dram_tensor("out", [BL, K, N, 1], F32, kind="ExternalOutput").ap()

    MAX = mybir.AluOpType.max
    MULT = mybir.AluOpType.mult
    BYPASS = mybir.AluOpType.bypass

    with tile.TileContext(nc) as tc:
        with tc.tile_pool(name="const", bufs=1) as cpool, \
             tc.tile_pool(name="wq", bufs=1) as wqpool, \
             tc.tile_pool(name="wv", bufs=1) as wvpool, \
             tc.tile_pool(name="vtp", bufs=2) as vtpool, \
             tc.tile_pool(name="work", bufs=3) as work, \
             tc.tile_pool(name="small", bufs=1) as small, \
             tc.tile_pool(name="dram", bufs=1, space="DRAM") as dpool, \
             tc.tile_pool(name="psum", bufs=4, space="PSUM") as pspool:

            # ---- DMA issue order is tuned so the PE can start at ~3us:
            # sync ring: tiny consts, then v chunk 0, then oneh, then v 1..8
            # scalar ring: the 16 W1v chunks (first chunks land first)
            # gpsimd (SWDGE): qt + W1q — only needed ~20us in (hq is deferred
            # until after the first chunk's v-matmuls)
            w2b_s = cpool.tile([128, HID], F32)
            nc.sync.dma_start(out=w2b_s, in_=w2b)
            ones_s = cpool.tile([1, BK], MM_DT)
            nc.sync.dma_start(out=ones_s, in_=ones)
            b1_s = cpool.tile([1, HID], MM_DT)
            nc.sync.dma_start(out=b1_s, in_=b1r)
            selb_s = cpool.tile([BK, BL], F32)
            nc.sync.dma_start(out=selb_s, in_=selb)
            selbt_s = cpool.tile([BL, BK], F32)
            nc.sync.dma_start(out=selbt_s, in_=selbt)

            def vt_chunk_tiles(ch):
                tiles = []
                for j in range(CC // VSPLIT):
                    t = vtpool.tile([128, VSPLIT, RC], MM_DT, tag=f"vt{j}")
                    nc.sync.dma_start(
                        out=t, in_=vt[ch, :, j * VSPLIT:(j + 1) * VSPLIT, :])
                    tiles.append(t)
                return tiles

            vt_cur = vt_chunk_tiles(0)

            oneh_s = cpool.tile([BK, R], MM_DT)
            nc.sync.dma_start(out=oneh_s, in_=oneh)

            wv_s = []
            for ccj in range(CC):
                t = wvpool.tile([128, HID], MM_DT, tag=f"wv{ccj}")
                nc.scalar.dma_start(out=t, in_=w1vt[:, ccj, :])
                wv_s.append(t)

            qt_s = cpool.tile([128, QC, BK], MM_DT)
            nc.gpsimd.dma_start(out=qt_s, in_=qt)
            wq_s = []
            for j in range(2):
                t = wqpool.tile([128, QC // 2, HID], MM_DT, tag=f"wq{j}")
                nc.gpsimd.dma_start(
                    out=t, in_=w1qt[:, j * (QC // 2):(j + 1) * (QC // 2), :])
                wq_s.append(t)

            # per-row logits, laid out [p, rt] with row = rt*128 + p.
            # 32 columns (27 used) so StreamTranspose's 32x32 blocks fit.
            ls_s = cpool.tile([128, 32], F32)
            nc.vector.memset(ls_s, 0.0)

            hq_s = cpool.tile([BK, HID], MM_DT)

            def emit_vmms(t, ps):
                for cc in range(CC):
                    lhsT = vt_cur[cc // VSPLIT][:, cc % VSPLIT:cc % VSPLIT + 1,
                                                t * RT:(t + 1) * RT]
                    mm(ps[:, 0:512], lhsT, wv_s[cc][:, 0:512],
                       start=(cc == 0), stop=False)
                    mm(ps[:, 512:1024], lhsT, wv_s[cc][:, 512:1024],
                       start=(cc == 0), stop=False)

            def emit_closer(rt, ps):
                oh = oneh_s[:, rt * RT:(rt + 1) * RT]
                mm(ps[:, 0:512], oh, hq_s[:, 0:512], start=False, stop=True)
                mm(ps[:, 512:1024], oh, hq_s[:, 512:1024], start=False, stop=True)
                relu_w2 = work.tile([128, HID], F32, tag="relu_w2")
                nc.vector.scalar_tensor_tensor(
                    out=relu_w2, in0=ps, scalar=0.0, in1=w2b_s,
                    op0=MAX, op1=MULT,
                    accum_out=ls_s[:, rt:rt + 1])

            # ---- chunk 0: v-matmuls for tiles 0..2 first, then hq (its DMAs
            # arrive under the v work), then the deferred closers.
            ps0 = []
            for t in range(NT):
                ps = pspool.tile([128, HID], F32, tag="ps", bufs=3)
                emit_vmms(t, ps)
                ps0.append(ps)

            # hq[bk, h] = q @ W1q^T + b1 (PSUM halves live in the small pool)
            hq_ps = [pspool.tile([BK, 512], F32, tag="sm", bufs=2,
                                 name=f"hq_ps{i}") for i in range(2)]
            for half in range(2):
                hs = slice(half * 512, (half + 1) * 512)
                for cq in range(QC):
                    mm(hq_ps[half],
                       qt_s[:, cq:cq + 1, :],
                       wq_s[cq // (QC // 2)][:, cq % (QC // 2):cq % (QC // 2) + 1, hs],
                       start=(cq == 0), stop=False)
                mm(hq_ps[half], ones_s, b1_s[:, hs], start=False, stop=True)
                nc.scalar.copy(hq_s[:, hs], hq_ps[half])

            for t in range(NT):
                emit_closer(t, ps0[t])
            vt_cur = vt_chunk_tiles(1)

            # ---- chunks 1..8
            for ch in range(1, NCH):
                for t in range(NT):
                    rt = ch * NT + t
                    ps = pspool.tile([128, HID], F32, tag="ps", bufs=3)
                    emit_vmms(t, ps)
                    emit_closer(rt, ps)
                if ch + 1 < NCH:
                    vt_cur = vt_chunk_tiles(ch + 1)

            # ---- logits [128, 32] -> linear DRAM (r = rt*128 + p)
            ls_t = cpool.tile([128, 32], F32)
            nc.vector.transpose(ls_t, ls_s)   # 32x32 block transposes
            lg = dpool.tile([R], F32)
            lg2 = lg.rearrange("(t p) -> t p", t=NRT, p=128)
            for i in range(4):
                # ls_t[32*i + t, p_lo] = L[t*128 + 32*i + p_lo], t in 0..26
                nc.sync.dma_start(
                    out=lg2[:, 32 * i:32 * i + 32],
                    in_=ls_t[32 * i:32 * i + NRT, :])

            # ---- softmax over k in [bk, n] layout
            s96 = small.tile([BK, N], F32)
            nc.sync.dma_start(out=s96, in_=lg.rearrange("(bk n) -> bk n", n=N))
            e96 = small.tile([BK, N], F32)
            nc.scalar.activation(e96, s96, mybir.ActivationFunctionType.Exp)
            sums_ps = pspool.tile([BL, N], F32, tag="sm", bufs=2)
            mm(sums_ps, selb_s, e96, start=True, stop=True)
            rcp = small.tile([BL, N], F32)
            nc.vector.reciprocal(rcp, sums_ps)
            rexp_ps = pspool.tile([BK, N], F32, tag="sm", bufs=2)
            mm(rexp_ps, selbt_s, rcp, start=True, stop=True)
            w96 = small.tile([BK, N], F32)
            nc.vector.scalar_tensor_tensor(
                out=w96, in0=e96, scalar=0.0, in1=rexp_ps,
                op0=BYPASS, op1=MULT)
            nc.sync.dma_start(
                out=out.rearrange("b k n o -> (b k) (n o)"), in_=w96)

    nc.compile()
    return nc


def _get_nc():
    global _NC_CACHE
    if _NC_CACHE is None:
        _NC_CACHE = _build_nc()
    return _NC_CACHE


def _prepare_in_maps(inputs):
    v = np.asarray(inputs["v"], dtype=np.float32)
    q = np.asarray(inputs["q"], dtype=np.float32)
    W1 = np.asarray(inputs["W1"], dtype=np.float32)
    g1 = np.float64(np.asarray(inputs["g1"]))
    b1 = np.asarray(inputs["b1"], dtype=np.float32)
    W2 = np.asarray(inputs["W2"], dtype=np.float32)
    g2 = np.float64(np.asarray(inputs["g2"]))
    # b2 is a scalar added to every logit -> cancels in softmax over k.

    W1e = ((g1 / np.linalg.norm(W1.astype(np.float64))) * W1).astype(np.float32)
    W2e = ((g2 / np.linalg.norm(W2.astype(np.float64))) * W2).astype(np.float32)

    # partition-major repacks: [..., 128 p, chunk, inner]
    w1vt = np.ascontiguousarray(                       # [128, 16, 1024]
        W1e[:, :VD].T.reshape(CC, 128, HID).transpose(1, 0, 2))
    w1qt = np.ascontiguousarray(                       # [128, 8, 1024]
        W1e[:, VD:].T.reshape(QC, 128, HID).transpose(1, 0, 2))
    w2b = np.ascontiguousarray(
        np.broadcast_to(W2e.reshape(1, HID), (128, HID)))
    b1r = np.ascontiguousarray(b1.reshape(1, HID))
    ones = np.ones((1, BK), dtype=np.float32)
    r = np.arange(R)
    oneh = (np.arange(BK)[:, None] == (r // N)[None, :]).astype(np.float32)
    selb = (np.arange(BL)[None, :] == (np.arange(BK) // K)[:, None]).astype(np.float32)
    selbt = np.ascontiguousarray(selb.T)

    shared = dict(w1vt=w1vt, w1qt=w1qt, w2b=w2b, b1r=b1r, ones=ones,
                  oneh=oneh, selb=selb, selbt=selbt)
    in_maps = []
    for c in range(NCORES):
        vl = v[c * BL:(c + 1) * BL].reshape(R, VD)
        # vt[ch, p, cc, r_in_chunk] = v[ch*RC + r, cc*128 + p]
        vt4 = np.ascontiguousarray(
            vl.T.reshape(CC, 128, NCH, RC).transpose(2, 1, 0, 3))
        ql = q[c * BL:(c + 1) * BL].reshape(BK, QD)
        qt3 = np.ascontiguousarray(                    # [128, 8, 96]
            ql.T.reshape(QC, 128, BK).transpose(1, 0, 2))
        in_maps.append(dict(vt=vt4, qt=qt3, **shared))
    return in_maps


def kernel(**inputs) -> np.ndarray:
    in_maps = _prepare_in_maps(inputs)
    nc = _get_nc()
    res = run_bass_kernel_spmd(nc, in_maps, list(range(NCORES)))
    outs = [res.results[c]["out"].reshape(BL, K, N, 1) for c in range(NCORES)]
    return np.concatenate(outs, axis=0)
